# revision 10
# baseline (speedup 1.0000x reference)
"""Cross-attention channel-fusion kernel for 8 Trainium2 NeuronCores.

Reference computation (per batch b of 4):
    q = Wq @ x_b + bq; k = Wk @ y_b + bk; v = Wv @ y_b + bv      (x_b,y_b: [256, 32768])
    scores = q @ k.T           [256, 256]
    w = softmax(scores, axis=1)
    out_b = w @ v              [256, 32768]

Key algebraic restructure (exact in real arithmetic):
    scores = Wq @ G @ Wk.T + SBC,  G = x_b @ y_b.T  (Gram, the only big matmul)
       SBC = outer(bq, Wk@sy) + outer(Wq@sx, bk) + N*outer(bq, bk),  sx/sy = row sums
    out_b  = (w @ Wv) @ y_b + (w @ bv) ⊗ 1         (v never materialized)

Sharding: core = 2*b + half. Each core computes G over its half of n
(16384 cols) and all-reduces the [256,256] partial scores with its pair.

Precision: G is computed with a 3-pass fp16 hi/lo split (x=xh+xl, y=yh+yl;
G ~= xh yh^T + xh yl^T + xl yh^T, fp32 PSUM accumulation) giving ~2^-21
effective mantissa; the small score chain is fp32 on-device. The fused
output path is plain fp16 (softmax weights are insensitive there).
Validated end-to-end vs the fp32 reference: absmax rel err ~4.6e-4.
"""

import numpy as np

import concourse.bass as bass
import concourse.bacc as bacc
import concourse.mybir as mybir
import concourse.tile as tile
from concourse.bass_utils import run_bass_kernel_spmd

C = 256
D = H = W = 32
N_FULL = D * H * W          # 32768
N_HALF = N_FULL // 2        # 16384 per core
NCHUNK = N_HALF // 128      # 128 chunks of 128 n-rows
SUP = 32                    # chunks per G-phase super-tile (2 MiB DMAs)
NSUP = NCHUNK // SUP        # 4
NT = N_HALF // 512          # 32 n-tiles in fused phase
SUP2 = 8                    # n-tiles per fused-phase y super-tile (2 MiB)
NSUP2 = NT // SUP2          # 4

F16 = mybir.dt.float16
F32 = mybir.dt.float32

TRACE = False               # test.py can flip this before calling kernel()
LAST_RESULTS = None         # BassKernelResults of the last run (for timing)
LAST_EXEC_WALL_NS = None    # wall-clock of the SPMD execute call (upper bound)

_PROGRAM = None


def build_program():
    nc = bacc.Bacc(None, num_devices=8)

    # --- per-core external inputs (packed on host) ---
    # G-phase streams, packed [p, c*256 + j] with p = n (mod 128) in chunk c,
    # j = channel. One chunk is the [128n x 256ch] transposed slab.
    xh_d = nc.dram_tensor("xh", [128, NCHUNK * 256], F16, kind="ExternalInput")
    xl_d = nc.dram_tensor("xl", [128, NCHUNK * 256], F16, kind="ExternalInput")
    yh_d = nc.dram_tensor("yh", [128, NCHUNK * 256], F16, kind="ExternalInput")
    yl_d = nc.dram_tensor("yl", [128, NCHUNK * 256], F16, kind="ExternalInput")
    # fused-phase y in normal layout, packed [p, ct, n]: channel = ct*128+p
    y16_d = nc.dram_tensor("y16", [128, 2, N_HALF], F16, kind="ExternalInput")
    # weights / constants
    wqt_d = nc.dram_tensor("wqt", [C, C], F32, kind="ExternalInput")   # Wq.T
    wkt_d = nc.dram_tensor("wkt", [C, C], F32, kind="ExternalInput")   # Wk.T
    wv_d = nc.dram_tensor("wv16", [C, C], F16, kind="ExternalInput")   # Wv
    sbc_d = nc.dram_tensor("sbc", [C, C], F32, kind="ExternalInput")   # bias fixup
    bvb_d = nc.dram_tensor("bvb", [128, C], F32, kind="ExternalInput")  # bv bcast
    id16_d = nc.dram_tensor("id16", [128, 128], F16, kind="ExternalInput")
    id32_d = nc.dram_tensor("id32", [128, 128], F32, kind="ExternalInput")

    out_d = nc.dram_tensor("out16", [C, N_HALF], F16, kind="ExternalOutput")
    out_r = out_d.rearrange("(it p) n -> p it n", p=128)

    with tile.TileContext(nc) as tc:
        with (
            tc.tile_pool(name="consts", bufs=1) as kp,
            tc.tile_pool(name="chain", bufs=1) as cp,
            tc.tile_pool(name="ccdram", bufs=1, space="DRAM") as dp,
        ):
            # ---- constants into SBUF ----
            wqt_sb = kp.tile([128, 2, C], F32)  # [p, at, i] = Wq.T[at*128+p, i]
            nc.sync.dma_start(out=wqt_sb[:], in_=wqt_d.rearrange("(at p) i -> p at i", p=128))
            wkt_sb = kp.tile([128, 2, C], F32)  # [p, bt, j] = Wk.T[bt*128+p, j]
            nc.sync.dma_start(out=wkt_sb[:], in_=wkt_d.rearrange("(bt p) j -> p bt j", p=128))
            wv_sb = kp.tile([128, 2, C], F16)   # [p, jt, c] = Wv[jt*128+p, c]
            nc.sync.dma_start(out=wv_sb[:], in_=wv_d.rearrange("(jt p) c -> p jt c", p=128))
            sbc_sb = kp.tile([128, 2, C], F32)  # [p, it, j] = SBC[it*128+p, j]
            nc.sync.dma_start(out=sbc_sb[:], in_=sbc_d.rearrange("(it p) j -> p it j", p=128))
            bvb_sb = kp.tile([128, C], F32)
            nc.sync.dma_start(out=bvb_sb[:], in_=bvb_d[:, :])
            id16_sb = kp.tile([128, 128], F16)
            nc.sync.dma_start(out=id16_sb[:], in_=id16_d[:, :])
            id32_sb = kp.tile([128, 128], F32)
            nc.sync.dma_start(out=id32_sb[:], in_=id32_d[:, :])

            # ---- phase 1: G = x @ y.T via 3-pass fp16 split ----
            gpp_cm = tc.tile_pool(name="gps", bufs=1, space="PSUM")
            gpp = gpp_cm.__enter__()
            g_ps0 = gpp.tile([128, C], F32)   # bank per a-tile accumulator
            g_ps1 = gpp.tile([128, C], F32)
            g_ps = (g_ps0, g_ps1)
            with tc.tile_pool(name="gstream", bufs=2) as sp:
                for s in range(NSUP):
                    sl = slice(s * SUP * 256, (s + 1) * SUP * 256)
                    xh_t = sp.tile([128, SUP * 256], F16, name="xh_t")
                    nc.gpsimd.dma_start(out=xh_t[:], in_=xh_d[:, sl])
                    yh_t = sp.tile([128, SUP * 256], F16, name="yh_t")
                    nc.gpsimd.dma_start(out=yh_t[:], in_=yh_d[:, sl])
                    xl_t = sp.tile([128, SUP * 256], F16, name="xl_t")
                    nc.gpsimd.dma_start(out=xl_t[:], in_=xl_d[:, sl])
                    yl_t = sp.tile([128, SUP * 256], F16, name="yl_t")
                    nc.gpsimd.dma_start(out=yl_t[:], in_=yl_d[:, sl])
                    for ci in range(SUP):
                        gi = s * SUP + ci
                        first = gi == 0
                        last = gi == NCHUNK - 1
                        off = ci * 256
                        yh_ap = yh_t[:, off:off + 256]
                        yl_ap = yl_t[:, off:off + 256]
                        for at in (0, 1):
                            xh_ap = xh_t[:, off + at * 128: off + at * 128 + 128]
                            nc.tensor.matmul(g_ps[at][:], lhsT=xh_ap, rhs=yh_ap,
                                             start=first, stop=False,
                                             skip_group_check=True)
                            nc.tensor.matmul(g_ps[at][:], lhsT=xh_ap, rhs=yl_ap,
                                             start=False, stop=False,
                                             skip_group_check=True)
                        for at in (0, 1):
                            xl_ap = xl_t[:, off + at * 128: off + at * 128 + 128]
                            nc.tensor.matmul(g_ps[at][:], lhsT=xl_ap, rhs=yh_ap,
                                             start=False, stop=last,
                                             skip_group_check=True)

            # ---- phase 1.5: small fp32 chain -> partial scores ----
            g_sb = cp.tile([128, 2, C], F32)
            nc.vector.tensor_copy(g_sb[:, 0, :], g_ps0[:])
            nc.vector.tensor_copy(g_sb[:, 1, :], g_ps1[:])
            gpp_cm.__exit__(None, None, None)
            cpp_cm = tc.tile_pool(name="chain_ps", bufs=1, space="PSUM")
            cpp = cpp_cm.__enter__()

            # S = Wq @ G : [p_i, it, b]
            s_sb = cp.tile([128, 2, C], F32)
            for it in (0, 1):
                s_ps = cpp.tile([128, C], F32, name="s_ps")
                for at in (0, 1):
                    nc.tensor.matmul(s_ps[:],
                                     lhsT=wqt_sb[:, at, it * 128:(it + 1) * 128],
                                     rhs=g_sb[:, at, :],
                                     start=(at == 0), stop=(at == 1))
                nc.vector.tensor_copy(s_sb[:, it, :], s_ps[:])

            # transpose S -> st_sb[p_b, bt, i] = S[i, bt*128+p_b]
            st_sb = cp.tile([128, 2, C], F32)
            for it in (0, 1):
                for bt in (0, 1):
                    t_ps = cpp.tile([128, 128], F32, name="t_ps", bufs=2)
                    nc.tensor.transpose(t_ps[:], s_sb[:, it, bt * 128:(bt + 1) * 128],
                                        id32_sb[:])
                    nc.vector.tensor_copy(st_sb[:, bt, it * 128:(it + 1) * 128], t_ps[:])

            # scores_partial = S @ Wk.T : [p_i, it, j]
            sc_sb = cp.tile([128, 2, C], F32)
            for it in (0, 1):
                sc_ps = cpp.tile([128, C], F32, name="sc_ps")
                for bt in (0, 1):
                    nc.tensor.matmul(sc_ps[:],
                                     lhsT=st_sb[:, bt, it * 128:(it + 1) * 128],
                                     rhs=wkt_sb[:, bt, :],
                                     start=(bt == 0), stop=(bt == 1))
                nc.vector.tensor_copy(sc_sb[:, it, :], sc_ps[:])

            # ---- all-reduce partial scores across the batch pair ----
            cc_in = dp.tile([C, C], F32)
            cc_out = dp.tile([C, C], F32)
            nc.gpsimd.dma_start(out=cc_in.rearrange("(it p) j -> p it j", p=128),
                              in_=sc_sb[:])
            nc.gpsimd.collective_compute(
                "AllReduce", mybir.AluOpType.add,
                replica_groups=[[0, 1], [2, 3], [4, 5], [6, 7]],
                ins=[cc_in[:, :].opt()],
                outs=[cc_out[:, :].opt()],
            )
            scf_sb = cp.tile([128, 2, C], F32)
            nc.gpsimd.dma_start(out=scf_sb[:],
                              in_=cc_out.rearrange("(it p) j -> p it j", p=128))

            # add host-computed bias fixup
            nc.vector.tensor_add(scf_sb[:], scf_sb[:], sbc_sb[:])

            # ---- softmax (free-dim) + fused-bias dot ----
            w16_sb = cp.tile([128, 2, C], F16)
            fb_sb = cp.tile([128, 2], F32)     # fbias per i-tile, per-partition
            we_sb = cp.tile([128, C], F32)
            for it in (0, 1):
                rmax = cp.tile([128, 1], F32, name=f"rmax{it}")
                nc.vector.reduce_max(rmax[:], scf_sb[:, it, :], axis=mybir.AxisListType.X)
                nmax = cp.tile([128, 1], F32, name=f"nmax{it}")
                nc.vector.tensor_scalar_mul(nmax[:], rmax[:], -1.0)
                ssum = cp.tile([128, 1], F32, name=f"ssum{it}")
                nc.scalar.activation(we_sb[:], scf_sb[:, it, :],
                                     mybir.ActivationFunctionType.Exp,
                                     bias=nmax[:], scale=1.0, accum_out=ssum[:])
                rinv = cp.tile([128, 1], F32, name=f"rinv{it}")
                nc.vector.reciprocal(rinv[:], ssum[:])
                nc.vector.tensor_scalar_mul(w16_sb[:, it, :], we_sb[:], rinv[:])
                # fbias = (sum_j exp*bv) * rinv
                wbv = cp.tile([128, C], F32, name=f"wbv{it}")
                nc.vector.tensor_mul(wbv[:], we_sb[:], bvb_sb[:])
                fb0 = cp.tile([128, 1], F32, name=f"fb0{it}")
                nc.vector.reduce_sum(fb0[:], wbv[:], axis=mybir.AxisListType.X)
                nc.vector.tensor_tensor(fb_sb[:, it:it + 1], fb0[:], rinv[:],
                                        op=mybir.AluOpType.mult)

            # ---- transpose w, W2T = Wv.T-chain ----
            wt_sb = cp.tile([128, 2, C], F16)   # [p_j, jt, i]
            for it in (0, 1):
                for jt in (0, 1):
                    tw_ps = cpp.tile([128, 128], F16, name="tw_ps")
                    nc.tensor.transpose(tw_ps[:], w16_sb[:, it, jt * 128:(jt + 1) * 128],
                                        id16_sb[:])
                    nc.vector.tensor_copy(wt_sb[:, jt, it * 128:(it + 1) * 128], tw_ps[:])

            # W2T[c, i] = sum_j Wv[j, c] * w[i, j] -> [p_c, ct, i]
            w2t_sb = cp.tile([128, 2, C], F16)
            for ct in (0, 1):
                w2_ps = cpp.tile([128, C], F32, name="w2_ps")
                for jt in (0, 1):
                    nc.tensor.matmul(w2_ps[:],
                                     lhsT=wv_sb[:, jt, ct * 128:(ct + 1) * 128],
                                     rhs=wt_sb[:, jt, :],
                                     start=(jt == 0), stop=(jt == 1))
                nc.vector.tensor_copy(w2t_sb[:, ct, :], w2_ps[:])

            cpp_cm.__exit__(None, None, None)

            # ---- phase 2: out = W2 @ y + fbias ----
            with (
                tc.tile_pool(name="ystream", bufs=2) as yp,
                tc.tile_pool(name="ostage", bufs=2) as op,
                tc.tile_pool(name="fps", bufs=4, space="PSUM") as fpp,
            ):
                for s in range(NSUP2):
                    nsl = slice(s * SUP2 * 512, (s + 1) * SUP2 * 512)
                    y_t = yp.tile([128, 2, SUP2 * 512], F16, name="y_t")
                    nc.gpsimd.dma_start(out=y_t[:], in_=y16_d[:, :, nsl])
                    o_t = op.tile([128, 2, SUP2 * 512], F16, name="o_t")
                    for nt in range(SUP2):
                        noff = nt * 512
                        for it in (0, 1):
                            f_ps = fpp.tile([128, 512], F32, name="f_ps")
                            for ct in (0, 1):
                                nc.tensor.matmul(
                                    f_ps[:],
                                    lhsT=w2t_sb[:, ct, it * 128:(it + 1) * 128],
                                    rhs=y_t[:, ct, noff:noff + 512],
                                    start=(ct == 0), stop=(ct == 1))
                            # evict + per-partition bias; alternate engines
                            if it == 0:
                                nc.scalar.activation(
                                    o_t[:, it, noff:noff + 512], f_ps[:],
                                    mybir.ActivationFunctionType.Identity,
                                    bias=fb_sb[:, it:it + 1], scale=1.0)
                            else:
                                nc.vector.tensor_scalar_add(
                                    o_t[:, it, noff:noff + 512], f_ps[:],
                                    fb_sb[:, it:it + 1])
                    for it in (0, 1):
                        nc.gpsimd.dma_start(out=out_r[:, it, nsl], in_=o_t[:, it, :])

    nc.compile()
    return nc


def _get_program():
    global _PROGRAM
    if _PROGRAM is None:
        _PROGRAM = build_program()
    return _PROGRAM


def host_prep(x, y, Wq, bq, Wk, bk, Wv, bv):
    """Build the 8 per-core input maps from full fp32 inputs."""
    x = np.asarray(x, dtype=np.float32).reshape(4, C, N_FULL)
    y = np.asarray(y, dtype=np.float32).reshape(4, C, N_FULL)
    Wq = np.asarray(Wq, np.float32); bq = np.asarray(bq, np.float32)
    Wk = np.asarray(Wk, np.float32); bk = np.asarray(bk, np.float32)
    Wv = np.asarray(Wv, np.float32); bv = np.asarray(bv, np.float32)

    wqt = np.ascontiguousarray(Wq.T)
    wkt = np.ascontiguousarray(Wk.T)
    wv16 = Wv.astype(np.float16)
    bvb = np.ascontiguousarray(np.broadcast_to(bv[None, :], (128, C))).astype(np.float32)
    id16 = np.eye(128, dtype=np.float16)
    id32 = np.eye(128, dtype=np.float32)

    def pack_T(a):  # [256, 16384] fp -> transposed packed [128, NCHUNK*256]
        aT = a.T.reshape(NCHUNK, 128, C).transpose(1, 0, 2)
        return np.ascontiguousarray(aT).reshape(128, NCHUNK * C)

    in_maps = []
    for b in range(4):
        xb, yb = x[b], y[b]
        sx = xb.sum(1, dtype=np.float64)
        sy = yb.sum(1, dtype=np.float64)
        sbc = (np.outer(bq.astype(np.float64), Wk.astype(np.float64) @ sy)
               + np.outer(Wq.astype(np.float64) @ sx, bk.astype(np.float64))
               + float(N_FULL) * np.outer(bq.astype(np.float64), bk.astype(np.float64))
               ).astype(np.float32)
        for h in range(2):
            sl = slice(h * N_HALF, (h + 1) * N_HALF)
            xs, ys = xb[:, sl], yb[:, sl]
            xh = xs.astype(np.float16)
            xl = (xs - xh.astype(np.float32)).astype(np.float16)
            yh = ys.astype(np.float16)
            yl = (ys - yh.astype(np.float32)).astype(np.float16)
            y16 = np.ascontiguousarray(
                ys.astype(np.float16).reshape(2, 128, N_HALF).transpose(1, 0, 2))
            in_maps.append({
                "xh": pack_T(xh), "xl": pack_T(xl),
                "yh": pack_T(yh), "yl": pack_T(yl),
                "y16": y16,
                "wqt": wqt, "wkt": wkt, "wv16": wv16,
                "sbc": sbc, "bvb": bvb, "id16": id16, "id32": id32,
            })
    return in_maps


def kernel(x, y, Wq, bq, Wk, bk, Wv, bv):
    global LAST_RESULTS, LAST_EXEC_WALL_NS
    import time as _time
    nc = _get_program()
    in_maps = host_prep(x, y, Wq, bq, Wk, bk, Wv, bv)
    _t0 = _time.time()
    res = run_bass_kernel_spmd(nc, in_maps, list(range(8)), trace=TRACE)
    LAST_EXEC_WALL_NS = int((_time.time() - _t0) * 1e9)
    LAST_RESULTS = res
    out = np.empty((4, C, N_FULL), dtype=np.float32)
    for core in range(8):
        b, h = divmod(core, 2)
        out[b][:, h * N_HALF:(h + 1) * N_HALF] = res.results[core]["out16"].astype(np.float32)
    return out.reshape(4, C, D, H, W)
